# revision 1
# baseline (speedup 1.0000x reference)
"""Trainium2 Bass kernel: batched 3x3 Lorenz-Jacobian Taylor matrix exponential.

Math (truncated Taylor expm, rel err ~1.9e-3 vs float64 reference):
    y0 = A0*u + B0*x1 + C1*(u*x2)
    y1 = D0*u + E0*x1 + F1*(u*x2)
    y2 = G1*u^2 + H0*x2 - F1*(u*x1)

Per tile ([128, 3E] fp16 in/out, dense slices):
  DVE : Q = u*x2; P = u*x1; z0 = B0*x1+t0; y0 = C1*Q+z0
  Pool: vq = u*u; issues output DMAs
  ACT : t0 = A0*u; y1 = copy(ps1); y2 = copy(ps2)   (copies pipelined)
  PE  : ps1 = (D0 I)@u + (E0 I)@x1 + (F1 I)@Q
        ps2 = (H0 I)@x2 + (G1 I)@vq + (-F1 I)@P
"""

import numpy as np
from contextlib import ExitStack

import concourse.bass as bass
import concourse.tile as tile
import concourse.mybir as mybir
from concourse.bass_utils import run_bass_kernel_spmd

A0 = 0.8679133685333335
B0 = 0.1827780802666667
C1 = -0.0018440311802469136
D0 = 0.5117786247466667
E0 = 1.0324136407733333
F1 = -0.019630097558847738
G1 = 0.005163287304691359
H0 = 0.9480639384616735

NCORES = 8
E_DEF = 489
T_DEF = 4
B_IN = 2_000_000

F16 = mybir.dt.float16
F32 = mybir.dt.float32
MULT = mybir.AluOpType.mult
ADD = mybir.AluOpType.add
COPY = mybir.ActivationFunctionType.Copy

W_COEF = ("D0", "E0", "F1", "H0", "G1", "mF1")


def build_nc(E=E_DEF, T=T_DEF):
    assert E <= 512
    nc = bass.Bass("TRN2", target_bir_lowering=False, debug=False)

    x_d = nc.dram_tensor("x", [T, 128, 3 * E], F16, kind="ExternalInput").ap()
    w_d = nc.dram_tensor("w", [128, 6 * 128], F16, kind="ExternalInput").ap()
    y_d = nc.dram_tensor("y", [T, 128, 3 * E], F16, kind="ExternalOutput").ap()

    with tile.TileContext(nc) as tc, ExitStack() as ctx:
        wp = ctx.enter_context(tc.tile_pool(name="wp", bufs=1))
        xp = ctx.enter_context(tc.tile_pool(name="xp", bufs=4))
        pp = ctx.enter_context(tc.tile_pool(name="pp", bufs=3))
        psp = ctx.enter_context(tc.psum_pool(name="psp", bufs=4))

        # tile 0's u-columns land first so compute can start ASAP
        Xs = [xp.tile([128, 3 * E], F16, tag="X", name=f"X{t}") for t in range(T)]
        Ys = [xp.tile([128, 3 * E], F16, tag="Y", name=f"Y{t}") for t in range(T)]
        nc.sync.dma_start(Xs[0][:, 0:E], x_d[0, :, 0:E])
        nc.sync.dma_start(Xs[0][:, E:3 * E], x_d[0, :, E:3 * E])
        nc.sync.dma_start(Xs[1][:, 0:E], x_d[1, :, 0:E])
        nc.sync.dma_start(Xs[1][:, E:3 * E], x_d[1, :, E:3 * E])
        W = wp.tile([128, 6 * 128], F16, tag="W", name="W")
        nc.sync.dma_start(W[:], w_d)
        for t in range(2, T):
            nc.sync.dma_start(Xs[t][:], x_d[t])
        wof = {nm: W[:, 128 * j:128 * (j + 1)] for j, nm in enumerate(W_COEF)}

        comp = [(Xs[t][:, 0:E], Xs[t][:, E:2 * E], Xs[t][:, 2 * E:3 * E])
                for t in range(T)]
        t0s, vqs, ps1s, ps2s = [], [], [], []
        for t in range(T):
            t0s.append(pp.tile([128, E], F16, tag="t0", name=f"t0_{t}"))
            vqs.append(pp.tile([128, E], F16, tag="vq", name=f"vq_{t}"))
            ps1s.append(psp.tile([128, E], F32, tag="ps1", name=f"ps1_{t}"))
            ps2s.append(psp.tile([128, E], F32, tag="ps2", name=f"ps2_{t}"))

        # ACT stream: t0(t) leads its tile; PSUM copies trail one tile behind
        def act_ops(t):
            u = comp[t][0]
            nc.scalar.activation(t0s[t][:], u, COPY, bias=0.0, scale=A0)

        def act_copies(t):
            nc.scalar.activation(Ys[t][:, E:2 * E], ps1s[t][:], COPY,
                                 bias=0.0, scale=1.0)
            nc.scalar.activation(Ys[t][:, 2 * E:3 * E], ps2s[t][:], COPY,
                                 bias=0.0, scale=1.0)

        # interleaved emission so no engine head-of-line-blocks on another
        pend_mms = []
        for t in range(T):
            u, x1, x2 = comp[t]
            Q = pp.tile([128, E], F16, tag="Q", name=f"Q{t}")
            P = pp.tile([128, E], F16, tag="P", name=f"P{t}")
            z0 = pp.tile([128, E], F16, tag="z0", name=f"z0_{t}")

            if t == 0:
                act_ops(0)
            nc.gpsimd.tensor_tensor(vqs[t][:], u, u, MULT)
            nc.vector.tensor_tensor(Q[:], u, x2, MULT)
            nc.vector.tensor_tensor(P[:], u, x1, MULT)
            if t + 1 < T:
                act_ops(t + 1)

            nc.tensor.matmul(ps1s[t][:], wof["D0"], u, start=True, stop=False)
            nc.tensor.matmul(ps1s[t][:], wof["E0"], x1, start=False, stop=False)
            nc.tensor.matmul(ps1s[t][:], wof["F1"], Q[:], start=False, stop=True)
            nc.tensor.matmul(ps2s[t][:], wof["H0"], x2, start=True, stop=False)
            nc.tensor.matmul(ps2s[t][:], wof["G1"], vqs[t][:], start=False, stop=False)
            nc.tensor.matmul(ps2s[t][:], wof["mF1"], P[:], start=False, stop=True)

            nc.vector.scalar_tensor_tensor(z0[:], x1, B0, t0s[t][:], MULT, ADD)
            nc.vector.scalar_tensor_tensor(Ys[t][:, 0:E], Q[:], C1, z0[:], MULT, ADD)
            act_copies(t)
            if t == T - 1:
                nc.sync.dma_start(y_d[t, :, 0:2 * E], Ys[t][:, 0:2 * E])
                nc.sync.dma_start(y_d[t, :, 2 * E:3 * E], Ys[t][:, 2 * E:3 * E])
            else:
                nc.sync.dma_start(y_d[t], Ys[t][:])

    _fix_tsp_waits(nc)
    return nc


def _fix_tsp_waits(nc):
    """Several TPB instruction encodings have a single sync-wait slot; Tile
    may attach several.  Hoist all-but-one onto same-engine nops."""
    eng_map = {
        mybir.EngineType.DVE: nc.vector,
        mybir.EngineType.Activation: nc.scalar,
        mybir.EngineType.Pool: nc.gpsimd,
        mybir.EngineType.PE: nc.tensor,
        mybir.EngineType.SP: nc.sync,
    }
    for blk in nc.m.functions[0].blocks:
        i = 0
        while i < len(blk.instructions):
            ins = blk.instructions[i]
            if ins.sync_info:
                waits = list(ins.sync_info.on_wait)
                if len(waits) > 1:
                    extra, keep = waits[:-1], waits[-1:]
                    ins.sync_info.on_wait = keep
                    for w in extra:
                        eng_map[ins.engine].nop()
                        nop = nc.m.functions[0].blocks[-1].instructions.pop()
                        assert isinstance(nop, mybir.InstNoOp)
                        nop.sync_info = mybir.SyncInfo(on_wait=[w], on_update=[])
                        blk.instructions.insert(i, nop)
                        i += 1
            i += 1


_CACHE = {}


def _get_nc(E=E_DEF, T=T_DEF):
    key = (E, T)
    if key not in _CACHE:
        _CACHE[key] = build_nc(E, T)
    return _CACHE[key]


def make_weights():
    w = np.zeros((128, 6 * 128), np.float16)
    idx = np.arange(128)
    for j, c in enumerate((D0, E0, F1, H0, G1, -F1)):
        w[idx, 128 * j + idx] = np.float16(c)
    return w


def prep_x(x, E=E_DEF, T=T_DEF):
    """[B,3] f32 -> [NCORES, T, 128, 3E] f16, components de-interleaved."""
    n_pc = 128 * E * T
    b_pad = NCORES * n_pc
    B = x.shape[0]
    xp = np.zeros((b_pad, 3), np.float16)
    xp[:B] = x.astype(np.float16)
    xr = (xp.reshape(NCORES, T, 128, E, 3)
            .transpose(0, 1, 2, 4, 3)
            .reshape(NCORES, T, 128, 3 * E))
    return np.ascontiguousarray(xr)


def unprep_y(ys, B, E=E_DEF, T=T_DEF):
    """list of per-core [T,128,3E] f16 -> [B,3] f32."""
    n_pc = 128 * E * T
    yr = (np.stack(ys, 0)
            .reshape(NCORES, T, 128, 3, E)
            .transpose(0, 1, 2, 4, 3)
            .reshape(NCORES * n_pc, 3))
    return np.ascontiguousarray(yr[:B]).astype(np.float32)


def kernel(x: np.ndarray) -> np.ndarray:
    E, T = E_DEF, T_DEF
    B = x.shape[0]
    assert x.shape[1] == 3 and NCORES * 128 * E * T >= B

    nc = _get_nc(E, T)
    shards = prep_x(x, E, T)
    w = make_weights()
    in_maps = [{"x": shards[c], "w": w} for c in range(NCORES)]
    res = run_bass_kernel_spmd(nc, in_maps, list(range(NCORES)))
    return unprep_y([r["y"] for r in res.results], B, E, T)

